# revision 2
# baseline (speedup 1.0000x reference)
"""Trainium2 Bass kernel for nn_MultiHeadAttention_47579647705431.

Multi-head attention (8 heads, dim 512, seq 1024, batch 16) with:
  - shared key/query linear (key_query_same=True: q and k both use Wk/bk)
  - causal (block-structured) mask
  - SimpleKT zero_pad: attention row 0 zeroed => out[:, 0, :] = bo (host)

Sharding: data-parallel over batch across 8 NeuronCores (2 batches/core).

v2 key idea (cost model charges matmuls by OUTPUT FREE SIZE only):
  AV uses the exp-block as the STATIONARY operand:
     av[s128, 64] += ex[t, s-block].T @ vp[t, 64]   (cost 64/block vs w)
  Denominators via 1-column ones matmuls (cost ~1 each) into a dedicated
  PSUM bank; normalization is a per-partition DVE tensor_scalar; the
  [s, d] concat tiles are PE-transposed back to [d, s] for the final
  projection (128 cycles per 128x128 tile).

Scheduling: need-driven work queue (kq half-groups / v-proj blocks) with
per-chunk deadlines popped one per j-iteration; flexible queue carries
transpose epilogues and final projections. PSUM: one pool, tags
st(2x2 banks) / av(2x1) / sh(1) / den(1) = 8 banks.

The walrus build here supports ONE sync wait per instruction; Tile emits
more. legalize_waits() hoists extra waits onto same-engine NoOps.
"""

import os
from contextlib import ExitStack

import numpy as np
import ml_dtypes

import concourse.bass as bass
import concourse.mybir as mybir
import concourse.tile as tile
from concourse.bass_utils import run_bass_kernel_spmd

F32 = mybir.dt.float32
BF16 = mybir.dt.bfloat16
BF = ml_dtypes.bfloat16

B, S, D, H, DH = 16, 1024, 512, 8, 64
NCORES = 8
BL = B // NCORES          # batches per core
N = BL * S                # tokens per core
NB = S // 128             # 128-blocks per sequence (8)
HP = H // 2               # head pairs
NEG = -1.0e9

LAST_SIM_NS = None
LAST_EXEC_NS = None


def legalize_waits(nc):
    """Split multi-wait instructions: keep one wait, hoist the rest onto
    preceding same-engine NoOps (this walrus encodes 1 wait/instruction)."""
    for f in nc.m.functions:
        for blk in f.blocks:
            il = blk.instructions
            i = 0
            while i < len(il):
                inst = il[i]
                si = inst.sync_info
                if si is not None and si.on_wait and len(si.on_wait) > 1:
                    waits = list(si.on_wait)
                    for j, w in enumerate(waits[:-1]):
                        nop = mybir.InstNoOp(
                            name=f"{inst.name}-hw{j}",
                            sync_info=mybir.SyncInfo(on_wait=[w], on_update=[]),
                            bass_nofuse=True,
                            engine=inst.engine,
                        )
                        il.insert(i, nop)
                        i += 1
                    si.on_wait = waits[-1:]
                i += 1


def _classify_mask(mask2d):
    """Classify 128x128 blocks of the [S, S] bool mask (query s, key t).

    Returns (status[j][i], patterns) in scores-transposed coords:
    j = key(t) block, i = query(s) block. status: -1 skip, -2 full,
    >=0 index into patterns (additive bf16 [t, s] blocks, 0 or NEG).
    """
    status = [[-1] * NB for _ in range(NB)]
    patterns = []
    pat_idx = {}
    for j in range(NB):
        for i in range(NB):
            blk = mask2d[i * 128:(i + 1) * 128, j * 128:(j + 1) * 128]  # [s, t]
            if blk.all():
                status[j][i] = -2
            elif not blk.any():
                status[j][i] = -1
            else:
                add = np.where(blk.T, 0.0, NEG).astype(BF)  # [t, s]
                key = add.tobytes()
                if key not in pat_idx:
                    pat_idx[key] = len(patterns)
                    patterns.append(add)
                status[j][i] = pat_idx[key]
    return status, patterns


def _plan_chunks(status, patterns):
    """Per (c, j): suffix run of non-skip query blocks within chunk c.

    Returns plan[c][j] = (w, mixes) where w = run width and mixes =
    [(col_offset_in_region, pattern_id), ...] for mixed blocks. Also
    first_j[c]. Asserts the suffix-nested structure the kernel relies on.
    """
    nch = S // 512
    plan = [[None] * NB for _ in range(nch)]
    first_j = [None] * nch
    for c in range(nch):
        i_lo, i_hi = 4 * c, 4 * c + 4
        prev_w = None
        for j in range(NB):
            sts = [status[j][i] for i in range(i_lo, i_hi)]
            nz = [k for k, s in enumerate(sts) if s != -1]
            if not nz:
                plan[c][j] = (0, [])
                continue
            if nz != list(range(nz[0], 4)):
                raise NotImplementedError("mask block structure not suffix-contiguous")
            w = 128 * len(nz)
            if prev_w is not None and w > prev_w:
                raise NotImplementedError("mask runs not nested over key blocks")
            prev_w = w
            mixes = [((k - nz[0]) * 128, sts[k]) for k in nz if sts[k] >= 0]
            plan[c][j] = (w, mixes)
            if first_j[c] is None:
                first_j[c] = j
    return plan, first_j


KNOB_POP_ALT = False
KNOB_SEQ = "phase"
KNOB_POPS_D = 1
KNOB_TAILACT = False


def _build(plan, first_j, nmix, has_bk, has_bv, has_bo):
    nc = bass.Bass()
    qt = nc.dram_tensor("qt", [128, 4, N], BF16, kind="ExternalInput")
    kt = nc.dram_tensor("kt", [128, 4, N], BF16, kind="ExternalInput")
    vt = nc.dram_tensor("vt", [128, 4, N], BF16, kind="ExternalInput")
    wkt = nc.dram_tensor("wkt", [128, 4, D], BF16, kind="ExternalInput")
    wvt = nc.dram_tensor("wvt", [128, 4, D], BF16, kind="ExternalInput")
    wot = nc.dram_tensor("wot", [128, 4, D], BF16, kind="ExternalInput")
    bk32 = nc.dram_tensor("bk32", [128, 4], F32, kind="ExternalInput")
    bvb = nc.dram_tensor("bvb", [1, D], BF16, kind="ExternalInput")
    bob = nc.dram_tensor("bob", [1, D], BF16, kind="ExternalInput")
    ident = nc.dram_tensor("ident", [128, 128], BF16, kind="ExternalInput")
    mixadd = nc.dram_tensor("mixadd", [max(nmix, 1), 128, 128], BF16,
                            kind="ExternalInput")
    out = nc.dram_tensor("out", [N, D], F32, kind="ExternalOutput")

    def mark(label):
        pass

    with tile.TileContext(nc) as tc:
        with ExitStack() as ctx:
            sing = ctx.enter_context(tc.tile_pool(name="sing", bufs=1))
            expp = ctx.enter_context(tc.tile_pool(name="expp", bufs=6))
            cnp = ctx.enter_context(tc.tile_pool(name="cnp", bufs=16))
            rcp = ctx.enter_context(tc.tile_pool(name="rcp", bufs=8))
            outp = ctx.enter_context(tc.tile_pool(name="outp", bufs=4))
            psp = ctx.enter_context(tc.tile_pool(name="psp", bufs=1, space="PSUM"))

            # ---- input loads: HWDGE queues only (sync/scalar/vector —
            # gpsimd DMA is software-DGE, ~1us of Pool engine per issue).
            # Critical-path order: wkt ob0 slice -> kt0 -> qt0 -> ident/mix
            # (first diagonal block) -> rest.
            wkt_sb = sing.tile([128, 4, D], BF16)
            nc.sync.dma_start(out=wkt_sb[:, :, 0:128], in_=wkt[:, :, 0:128])
            kt_c, qt_c, vt_c = [None] * 4, [None] * 4, [None] * 4

            def load_ch(ch):
                csl = slice(ch * 512, ch * 512 + 512)
                t = sing.tile([128, 4, 512], BF16, tag=f"ktc{ch}")
                nc.sync.dma_start(out=t, in_=kt[:, :, csl])
                kt_c[ch] = t
                t = sing.tile([128, 4, 512], BF16, tag=f"qtc{ch}")
                nc.scalar.dma_start(out=t, in_=qt[:, :, csl])
                qt_c[ch] = t

            def load_v(ch):
                csl = slice(ch * 512, ch * 512 + 512)
                t = sing.tile([128, 4, 512], BF16, tag=f"vtc{ch}")
                nc.sync.dma_start(out=t, in_=vt[:, :, csl])
                vt_c[ch] = t

            load_ch(0)
            ident_sb = sing.tile([128, 128], BF16)
            nc.sync.dma_start(out=ident_sb, in_=ident[:, :])
            mix_sb = sing.tile([128, max(nmix, 1), 128], BF16)
            nc.sync.dma_start(out=mix_sb, in_=mixadd.rearrange("m t s -> t m s"))
            wvt_sb = sing.tile([128, 4, D], BF16)
            nc.sync.dma_start(out=wvt_sb, in_=wvt[:, :, :])
            load_v(0)
            nc.sync.dma_start(out=wkt_sb[:, :, 128:512], in_=wkt[:, :, 128:512])
            load_ch(1)
            load_v(1)
            load_ch(2)
            load_v(2)
            load_ch(3)
            load_v(3)
            wot_sb = sing.tile([128, 4, D], BF16)
            nc.scalar.dma_start(out=wot_sb, in_=wot[:, :, :])
            bk_sb = None
            if has_bk:
                bk_sb = sing.tile([128, 4], F32)
                nc.sync.dma_start(out=bk_sb, in_=bk32[:, :])
            bvb_sb = bob_sb = ones_k1 = None
            if has_bv or has_bo:
                ones_k1 = sing.tile([1, 128], BF16)
                nc.vector.memset(ones_k1, 1.0)
            if has_bv:
                bvb_sb = sing.tile([1, D], BF16)
                nc.sync.dma_start(out=bvb_sb, in_=bvb[:, :])
            if has_bo:
                bob_sb = sing.tile([1, D], BF16)
                nc.sync.dma_start(out=bob_sb, in_=bob[:, :])
            ones_col = sing.tile([128, 1], BF16)
            nc.vector.memset(ones_col, 1.0)

            kp_sb = sing.tile([128, 4, N], BF16)
            qp_sb = sing.tile([128, 4, N], BF16)
            vp_sb = sing.tile([128, N // 128, 512], BF16)
            ct_sb = sing.tile([128, 4, N], BF16)
            den_t = psp.tile([128, 512], F32, tag="den", bufs=1)

            # ---- work units ----
            def kq_half(ob, ch, which, tag="sh", fast=False):
                mark("pre")
                csl = slice(ch * 512, ch * 512 + 512)
                ps = psp.tile([128, 512], F32, tag=tag, bufs=2 if tag == "av" else 1)
                src = kt_c[ch] if which == "k" else qt_c[ch]
                for db in range(4):
                    nc.tensor.matmul(
                        ps, wkt_sb[:, db, ob * 128:(ob + 1) * 128],
                        src[:, db, :], start=(db == 0), stop=(db == 3))
                dst = kp_sb if which == "k" else qp_sb
                if has_bk:
                    if which == "k":
                        nc.scalar.add(dst[:, ob, csl], ps, bk_sb[:, ob:ob + 1])
                    else:
                        nc.vector.tensor_scalar_add(
                            dst[:, ob, csl], ps, bk_sb[:, ob:ob + 1])
                elif which == "k":
                    nc.vector.tensor_copy(out=dst[:, ob, csl], in_=ps)
                else:
                    nc.vector.tensor_copy(out=dst[:, ob, csl], in_=ps)
                mark(f"kq{which}({ob},{ch})")

            def v_proj(nt, tag="sh"):
                mark("pre")
                psV = psp.tile([128, 512], F32, tag=tag, bufs=2 if tag == "av" else 1)
                for db in range(4):
                    nc.tensor.matmul(
                        psV, vt_c[nt // 4][:, db, (nt % 4) * 128:(nt % 4) * 128 + 128],
                        wvt_sb[:, db, :], start=(db == 0),
                        stop=(db == 3 and not has_bv))
                if has_bv:
                    nc.tensor.matmul(psV, ones_k1, bvb_sb[0:1, :],
                                     start=False, stop=True)
                nc.vector.tensor_copy(out=vp_sb[:, nt, :], in_=psV)
                mark(f"v({nt})")

            def emit_scores(b, hp, c, j, st, base=None, merged=False):
                """Write scores^T for block j into st.

                Singleton (2w can exceed 512): h0 at [512-w,512), h1 at
                [512,512+w) — each half sits in its own psum bank, start=True
                on both. Merged member (2w <= 512): the 2w region [base,
                base+2w) lies inside ONE bank; start=True only when
                first_in_bank.
                """
                mark("pre")
                w, mixes = plan[c][j]
                tsl = slice(b * S + j * 128, b * S + j * 128 + 128)
                ssl = slice(b * S + c * 512 + 512 - w,
                            b * S + c * 512 + 512)
                if not merged:
                    base = 512 - w
                    sA = sB = True
                else:
                    sA, sB = True, False
                nc.tensor.matmul(st[:, base:base + w],
                                 kp_sb[0:64, hp, tsl],
                                 qp_sb[0:64, hp, ssl],
                                 start=sA, stop=not mixes,
                                 skip_group_check=True)
                nc.tensor.matmul(st[:, base + w:base + 2 * w],
                                 kp_sb[64:128, hp, tsl],
                                 qp_sb[64:128, hp, ssl],
                                 start=sB, stop=not mixes,
                                 skip_group_check=True)
                for mi, (off, pid) in enumerate(mixes):
                    last = mi == len(mixes) - 1
                    nc.tensor.matmul(st[:, base + off:base + off + 128],
                                     ident_sb, mix_sb[:, pid, :],
                                     start=False, stop=last,
                                     skip_group_check=True)
                    nc.tensor.matmul(st[:, base + w + off:base + w + off + 128],
                                     ident_sb, mix_sb[:, pid, :],
                                     start=False, stop=last,
                                     skip_group_check=True)
                mark(f"scores({b},{hp},{c},{j})")

            def chunk_groups(c):
                """Partition the chunk's js into exp groups: consecutive
                pairs (ja, jb) merge when both 2w <= 512. Layout: ja
                end-aligned at 512, jb at 512. Returns list of
                [(j, base, ex_off), ...] per group."""
                js = [j for j in range(NB) if plan[c][j][0] > 0]
                groups = []
                i = 0
                nomerge = True
                while i < len(js):
                    j = js[i]
                    w = plan[c][j][0]
                    if (not nomerge and 2 * w <= 512 and i + 1 < len(js)
                            and 2 * plan[c][js[i + 1]][0] <= 512):
                        w2 = plan[c][js[i + 1]][0]
                        groups.append([(j, 512 - 2 * w, 0),
                                       (js[i + 1], 512, 2 * w)])
                        i += 2
                    else:
                        groups.append([(j, 512 - w, 0)])
                        i += 1
                return groups

            fin_ctr = [0]

            def final_group(b, nt, tag="sh"):
                mark("pre")
                gnt = b * NB + nt
                psO = psp.tile([128, 512], F32, tag=tag,
                               bufs=2 if tag == "av" else 1)
                for hp in range(4):
                    nc.tensor.matmul(
                        psO, ct_sb[:, hp, gnt * 128:(gnt + 1) * 128],
                        wot_sb[:, hp, :], start=(hp == 0),
                        stop=(hp == 3 and not has_bo))
                if has_bo:
                    nc.tensor.matmul(psO, ones_k1, bob_sb[0:1, :],
                                     start=False, stop=True)
                ot = outp.tile([128, 512], F32)
                if KNOB_TAILACT and b == 1 and nt >= 4:
                    nc.scalar.copy(ot, psO)
                    nc.scalar.dma_start(
                        out=out[gnt * 128:(gnt + 1) * 128, :], in_=ot)
                else:
                    nc.vector.tensor_copy(out=ot, in_=psO)
                    if nt % 2 == 0:
                        nc.scalar.dma_start(
                            out=out[gnt * 128:(gnt + 1) * 128, :], in_=ot)
                    else:
                        nc.sync.dma_start(
                            out=out[gnt * 128:(gnt + 1) * 128, :], in_=ot)
                mark(f"final({b},{nt})")

            # ---- need-driven schedule ----
            # seq: b=0 c=0 (ch0 only), b=1 c=0 (ch2), b=0 c=1 (ch1),
            # b=1 c=1 (ch3) — fastest path to first exp, late chunks
            # need late DMA.
            if KNOB_SEQ == "interleave":
                seq = [(0, 0, 0), (0, 1, 0), (0, 0, 1), (1, 0, 0),
                       (0, 1, 1), (0, 2, 0), (1, 0, 1), (0, 3, 0),
                       (0, 2, 1), (1, 1, 0), (0, 3, 1), (1, 2, 0),
                       (1, 1, 1), (1, 3, 0), (1, 2, 1), (1, 3, 1)]
            elif KNOB_SEQ == "phase":
                seq = ([(0, hp, 0) for hp in range(4)]
                       + [(1, hp, 0) for hp in range(4)]
                       + [(0, hp, 1) for hp in range(4)]
                       + [(1, hp, 1) for hp in range(4)])
            else:  # balt: b-alternating
                seq = [(0, 0, 0), (1, 0, 0), (0, 1, 0), (1, 1, 0),
                       (0, 2, 0), (1, 2, 0), (0, 3, 0), (1, 3, 0),
                       (0, 0, 1), (1, 0, 1), (0, 1, 1), (1, 1, 1),
                       (0, 2, 1), (1, 2, 1), (0, 3, 1), (1, 3, 1)]

            # workq: (deadline_chunk_index, fn) — must run before that chunk.
            # Deadlines are staggered a few chunks early so drains never
            # bunch at a prologue.
            workq = []
            for nt in (1, 2, 3):
                workq.append((0, lambda nt=nt: v_proj(nt)))
            for ci, (b_, hp_, c_) in enumerate(seq):
                if ci == 0:
                    continue
                # kq planes needed: (hp, 2b) for c=0; additionally
                # (hp, 2b+1) for c=1 — the 2b plane is covered by the c=0
                # chunk of the same (b, hp) earlier in seq.
                need_ch = 2 * b_ + c_
                workq.append((max(0, ci - 2),
                              lambda hp_=hp_, ch=need_ch: kq_half(hp_, ch, "k")))
                workq.append((max(0, ci - 1),
                              lambda hp_=hp_, ch=need_ch: kq_half(hp_, ch, "q")))
                first_bc = min(i for i, s in enumerate(seq)
                               if s[0] == b_ and s[2] == c_)
                if ci == first_bc:
                    if c_ == 0:
                        vts = list(range(b_ * NB, b_ * NB + 4))
                    else:
                        vts = list(range(b_ * NB + 4, b_ * NB + 8))
                    for vx, nt in enumerate(vts):
                        workq.append((max(0, ci - 2 + vx // 2),
                                      lambda nt=nt: v_proj(nt)))
            workq.sort(key=lambda it: it[0])
            flex = []

            cur_chunk = [0]

            pop_par = [0]

            def pop_filler():
                pop_par[0] ^= 1
                due = workq and workq[0][0] <= cur_chunk[0] + 1
                take_w = due and (not KNOB_POP_ALT or pop_par[0] or not flex)
                if take_w:
                    workq.pop(0)[1]()
                elif flex:
                    flex.pop(0)()
                elif workq:
                    workq.pop(0)[1]()

            def drain_needs(ci):
                while workq and workq[0][0] <= ci:
                    workq.pop(0)[1]()

            st_store = {}
            chunk_slot = {}

            def attention_c(ci, b, hp, c, next_start=None):
                fj = first_j[c]
                if fj is None:
                    return
                cur_chunk[0] = ci
                if ci > 0:
                    drain_needs(ci)
                slot = chunk_slot.setdefault((b, hp, c), len(chunk_slot)) * 8
                av = psp.tile([128, 512], F32, tag="av", bufs=2)
                av_first = [True]
                den_first = [True]
                js = [j for j in range(NB) if plan[c][j][0] > 0]
                last_cover = {}
                for j in js:
                    w, _ = plan[c][j]
                    for i_rel in range(4):
                        if 128 * i_rel >= 512 - w:
                            last_cover[i_rel] = j
                st_t = st_store.setdefault((b, hp, c), {})
                groups = chunk_groups(c)

                # deficit-based filler schedule: ACT exp time minus chunk's
                # own PE work, in ~850ns filler units, spread over groups
                exp_ns = sum(
                    sum(2 * plan[c][j][0] for j, _, _ in grp) * 0.833 + 242
                    for grp in groups)
                main_cy = sum(
                    2 * plan[c][j][0]
                    + sum(1 for i in range(4)
                          if 128 * i >= 512 - plan[c][j][0]) * 2 * 65
                    + len(plan[c][j][1]) * 256
                    for j in js)
                deficit = exp_ns - main_cy * 0.4167
                pops = max(0, min(int(round(deficit / 850.0)) + KNOB_POPS_D,
                                  2 * len(groups)))
                if ci == 0:
                    pops = max(pops, 5)
                pop_plan = [0] * len(groups)
                for k in range(pops):
                    pop_plan[(k * len(groups)) // pops] += 1

                def emit_group(gx):
                    stn = psp.tile([128, 1024], F32, tag="st", bufs=2)
                    st_t[gx] = stn
                    grp = groups[gx]
                    for j, base, _ in grp:
                        emit_scores(b, hp, c, j, stn, base=base,
                                    merged=len(grp) > 1)

                if 0 not in st_t:
                    emit_group(0)
                for gx, grp in enumerate(groups):
                    st = st_t.pop(gx)
                    lo = grp[0][1]
                    tot = sum(2 * plan[c][j][0] for j, _, _ in grp)
                    ex = expp.tile([128, 1024], BF16)
                    mark("pre")
                    nc.scalar.activation(
                        ex[:, 0:tot], st[:, lo:lo + tot],
                        mybir.ActivationFunctionType.Exp, scale=0.125)
                    mark(f"exp({b},{hp},{c},g{gx})")
                    if gx + 1 < len(groups):
                        emit_group(gx + 1)
                    elif next_start is not None:
                        next_start()
                    # fillers run between next scores and this group's AVs:
                    # gives the ACT exp time to finish so AVs never wait
                    for _ in range(pop_plan[gx]):
                        pop_filler()
                    for j, base, exo in grp:
                        w, _ = plan[c][j]
                        vrow = b * NB + j
                        for i_rel in range(4):
                            off = 128 * i_rel - (512 - w)
                            if off < 0:
                                continue
                            lastj = last_cover[i_rel] == j
                            for hh in range(2):
                                lhs = ex[:, exo + hh * w + off:
                                         exo + hh * w + off + 128]
                                nc.tensor.matmul(
                                    av[:, 256 * hh + 64 * i_rel:
                                       256 * hh + 64 * i_rel + 64],
                                    lhs,
                                    vp_sb[:, vrow,
                                          128 * hp + 64 * hh:
                                          128 * hp + 64 * hh + 64],
                                    start=av_first[0], stop=lastj,
                                    skip_group_check=True)
                                av_first[0] = False
                                nc.tensor.matmul(
                                    den_t[:, slot + 4 * hh + i_rel:
                                          slot + 4 * hh + i_rel + 1],
                                    lhs, ones_col,
                                    start=den_first[0], stop=lastj,
                                    skip_group_check=True)
                                den_first[0] = False
                        mark(f"AV({b},{hp},{c},{j})")

                rc = rcp.tile([128, 8], F32)
                nc.vector.reciprocal(out=rc, in_=den_t[:, slot:slot + 8])
                cn = cnp.tile([128, 4, 128], BF16)
                for i_rel in range(4):
                    for hh in range(2):
                        nc.vector.tensor_scalar_mul(
                            cn[:, i_rel, 64 * hh:64 * hh + 64],
                            av[:, 256 * hh + 64 * i_rel:
                               256 * hh + 64 * i_rel + 64],
                            rc[:, 4 * hh + i_rel:4 * hh + i_rel + 1])
                mark(f"norm({b},{hp},{c})")

                def epilogue(b=b, hp=hp, c=c, cn=cn):
                    # transpose [s, f] -> [f, s] per i-block, then one copy
                    tp = psp.tile([128, 512], BF16, tag="sh")
                    for i_rel in range(4):
                        nc.tensor.transpose(
                            tp[:, 128 * i_rel:128 * i_rel + 128],
                            cn[:, i_rel, :], ident_sb)
                    osl = slice(b * S + c * 512, b * S + c * 512 + 512)
                    nc.vector.tensor_copy(out=ct_sb[:, hp, osl], in_=tp)
                    mark(f"epi({b},{hp},{c})")

                flex.append(epilogue)

            def make_next_start(nci, nb_, nhp_, nc_):
                def _start():
                    drain_needs(nci)  # kq planes for the next chunk
                    grp = chunk_groups(nc_)[0]
                    st = psp.tile([128, 1024], F32, tag="st", bufs=2)
                    st_store.setdefault((nb_, nhp_, nc_), {})[0] = st
                    for j, base, _ in grp:
                        emit_scores(nb_, nhp_, nc_, j, st, base=base,
                                    merged=len(grp) > 1)
                return _start

            # ---- emission ----
            kq_half(0, 0, "k", tag="sh")
            kq_half(0, 0, "q", tag="av")
            v_proj(0, tag="av")

            done_count = {}
            for ci, (b_, hp_, c_) in enumerate(seq):
                nxt = (make_next_start(ci + 1, *seq[ci + 1])
                       if ci + 1 < len(seq) else None)
                attention_c(ci, b_, hp_, c_, next_start=nxt)
                key = (b_, c_)
                done_count[key] = done_count.get(key, 0) + 1
                if done_count[key] == 4:
                    lo = 4 * c_
                    for nt_ in range(lo, lo + 4):
                        def _fin(b_=b_, nt_=nt_, tag="sh"):
                            final_group(b_, nt_, tag=tag)
                        _fin.is_final = True
                        flex.append(_fin)
            if True:
                while workq:
                    workq.pop(0)[1]()
                leftovers = list(flex)
                flex.clear()
                for i, fn in enumerate(leftovers):
                    if getattr(fn, "is_final", False):
                        fn(tag="av" if i % 2 else "sh")
                    else:
                        fn()
            else:
                while workq or flex:
                    pop_filler()

    return nc


_prog_cache = {}


def kernel(q, k, v, mask, zero_pad, Wk, bk, Wv, bv, Wo, bo):
    global LAST_SIM_NS, LAST_EXEC_NS
    q = np.asarray(q, dtype=np.float32)
    k = np.asarray(k, dtype=np.float32)
    v = np.asarray(v, dtype=np.float32)
    Wk = np.asarray(Wk, dtype=np.float32)
    Wv = np.asarray(Wv, dtype=np.float32)
    Wo = np.asarray(Wo, dtype=np.float32)
    bk = np.asarray(bk, dtype=np.float32).reshape(D)
    bv = np.asarray(bv, dtype=np.float32).reshape(D)
    bo = np.asarray(bo, dtype=np.float32).reshape(D)
    mask2d = np.asarray(mask).reshape(S, S).astype(bool)
    zp = int(np.asarray(zero_pad))

    status, patterns = _classify_mask(mask2d)
    plan, first_j = _plan_chunks(status, patterns)
    nmix = len(patterns)
    has_bk = bool(np.any(bk))
    has_bv = bool(np.any(bv))
    has_bo = bool(np.any(bo))

    sig = (tuple(tuple(r) for r in status), nmix, has_bk, has_bv, has_bo)
    if sig not in _prog_cache:
        nc_new = _build(plan, first_j, nmix, has_bk, has_bv, has_bo)
        legalize_waits(nc_new)   # hardware-only pass
        _prog_cache[sig] = nc_new
    nc = _prog_cache[sig]

    def _sbuf_layout(wt):
        # [D, X] -> [128, 4, X]: row d = a*128+p  ->  [p, a, :]
        return np.ascontiguousarray(wt.reshape(4, 128, -1).transpose(1, 0, 2))

    wkt = _sbuf_layout(Wk.T.astype(BF))
    wvt = _sbuf_layout(Wv.T.astype(BF))
    wot = _sbuf_layout(Wo.T.astype(BF))
    bk32 = np.ascontiguousarray(bk.reshape(4, 128).T).astype(np.float32)
    bvb = bv.reshape(1, D).astype(BF)
    bob = bo.reshape(1, D).astype(BF)
    ident = np.eye(128, dtype=BF)
    mixadd = (np.stack(patterns) if patterns
              else np.zeros((1, 128, 128), np.float32)).astype(BF)

    common = dict(wkt=wkt, wvt=wvt, wot=wot, bk32=bk32, bvb=bvb, bob=bob,
                  ident=ident, mixadd=mixadd)
    in_maps = []
    for ci in range(NCORES):
        sl = slice(ci * BL, (ci + 1) * BL)
        in_maps.append(dict(
            qt=_sbuf_layout(q[sl].reshape(N, D).T.astype(BF)),
            kt=_sbuf_layout(k[sl].reshape(N, D).T.astype(BF)),
            vt=_sbuf_layout(v[sl].reshape(N, D).T.astype(BF)),
            **common))

    if os.environ.get("BASS_KERNEL_SIM_TIME"):
        from concourse.timeline_sim import TimelineSim
        LAST_SIM_NS = TimelineSim(nc).simulate()

    res = run_bass_kernel_spmd(nc, in_maps, list(range(NCORES)))
    LAST_EXEC_NS = res.exec_time_ns

    outs = [res.results[ci]["out"].reshape(BL, S, D) for ci in range(NCORES)]
    full = np.concatenate(outs, axis=0)
    if zp:
        full[:, 0, :] = bo
    return full
